# revision 1
# baseline (speedup 1.0000x reference)
"""Trainium2 Bass kernel for nn_GAT_Top (2-layer GAT + FC/BN + DistMult edge head).

Self-contained: takes FULL inputs, shards across 8 NeuronCores internally
(dst-node ownership for the sparse phases, node-parallel dense phases, halo
exchange via AllGather of node-feature tables), returns the FULL output.

v3: host-precomputed scatter matrices S/S^T streamed from DRAM (replaces
on-chip IS_EQ builds + PE transposes), fused num|den matmuls, block-wide
vector ops, 4 SWDGE queues with split gathers, chunked AllGathers overlapped
with producer compute (chunk-major table layout via host index remap).
"""
import os
import numpy as np
import ml_dtypes

import concourse.bass as bass
import concourse.bacc as bacc
import concourse.tile as tile
from concourse import mybir
from concourse.bass_utils import run_bass_kernel_spmd

F32 = mybir.dt.float32
BF16 = mybir.dt.bfloat16
I16 = mybir.dt.int16
AO = mybir.AluOpType
AF = mybir.ActivationFunctionType

# problem constants (hardcoded per harness contract)
N, E, TE = 10000, 160000, 65536
NCORES = 8
NPC = N // NCORES            # 1250 nodes per core
NBLK = 10                    # dst blocks of 128 per core (last has 98)
TEC = TE // NCORES           # 8192 train edges per core
F1, F2 = 384, 256
H1 = 8
C1 = 48
ROW1 = 512                   # hx1 row bf16: h(384) | asrc(8) | pad
ROW2 = 384                   # hx2 row bf16: h2(256) | asrc2(1) | one(1) | pad
NEG_SLOPE = 0.2
BN_EPS = 1e-5
PADLOC = 999.0
EC = 1024                    # train sub-chunk
RSP = 1250                   # AllGather chunk split disabled (single-writer rule)
CHUNK_AG = False

last_exec_time_ns = None
_PROG_CACHE = {}


def _wrap_idx(idx):
    """int idx [n] (n%16==0) -> int16 [128, n//16]: i at [i%16, i//16], replicated
    across the 8 groups of 16 partitions (gpsimd cores)."""
    n = idx.shape[0]
    w = np.asarray(idx, np.int16).reshape(n // 16, 16).T  # [16, n//16]
    return np.tile(w, (8, 1))


def _pack_slabs(w, nslab):
    """[K, M] -> [128, nslab, M] with [p, s, m] = w[s*128+p, m]."""
    K, M = w.shape
    out = np.zeros((128, nslab, M), w.dtype)
    for s in range(nslab):
        k0, k1 = s * 128, min((s + 1) * 128, K)
        out[: k1 - k0, s, :] = w[k0:k1, :]
    return out


def _pack_col(v, nslab):
    """[K] -> [128, nslab] with [p, s] = v[s*128+p]."""
    K = v.shape[0]
    out = np.zeros((128, nslab), v.dtype)
    for s in range(nslab):
        k0, k1 = s * 128, min((s + 1) * 128, K)
        out[: k1 - k0, s] = v[k0:k1]
    return out


def _remap_rows(n):
    """Global node id -> row in the AG-chunk-major gather tables.

    Tables are built by two chunked AllGathers: rows [0, 8*RSP) hold each
    core's local rows [0, RSP), rows [8*RSP, N) hold local rows [RSP, NPC).
    """
    c = n // NPC
    r = n % NPC
    return np.where(r < RSP, c * RSP + r,
                    8 * RSP + c * (NPC - RSP) + (r - RSP))


def _prepare(inputs):
    """Host-side preprocessing: edge partitioning/sorting/padding + weight packing."""
    ei = np.asarray(inputs["edge_index"]).astype(np.int64)
    loops = np.arange(N, dtype=np.int64)
    src = np.concatenate([ei[0], loops])
    dst = np.concatenate([ei[1], loops])

    per_core = []
    tpb_max = 1
    for c in range(NCORES):
        sel = (dst // NPC) == c
        s_c, d_c = src[sel], dst[sel]
        order = np.argsort(d_c, kind="stable")
        s_c, d_c = s_c[order], d_c[order]
        blocks = []
        base = c * NPC
        for b in range(NBLK):
            lo, hi = base + b * 128, min(base + (b + 1) * 128, base + NPC)
            m = (d_c >= lo) & (d_c < hi)
            bs, bd = s_c[m], d_c[m] - lo
            blocks.append((bs, bd))
            tpb_max = max(tpb_max, (len(bs) + 127) // 128)
        per_core.append(blocks)
    TPB = tpb_max + (tpb_max % 2)  # even, so gathers split in halves

    # per-core index blobs
    cores = []
    tid = np.asarray(inputs["train_edge_id"]).astype(np.int64)
    pair_nodes = _remap_rows(ei[:, tid])  # [2, TE] rows in ho table
    d_iota = np.arange(128)
    for c in range(NCORES):
        hx_idx = np.zeros((NBLK, TPB * 128), np.int64)
        dstloc = np.full((NBLK, TPB * 128), PADLOC, np.float32)
        for b, (bs, bd) in enumerate(per_core[c]):
            n = len(bs)
            hx_idx[b, :n] = _remap_rows(bs)
            dstloc[b, :n] = bd.astype(np.float32)
        hx_w = np.concatenate([_wrap_idx(hx_idx[b]) for b in range(NBLK)], axis=1)
        # scatter matrices S [edge part, (b,t,dst)] and St [dst part, (b,t,edge)]
        dl = dstloc.reshape(NBLK, TPB, 128)                  # [b, t, e]
        S4 = (dl[:, :, :, None] == d_iota[None, None, None, :])   # [b,t,e,d]
        S_blob = S4.transpose(2, 0, 1, 3).reshape(128, NBLK * TPB * 128)
        St_blob = S4.transpose(3, 0, 1, 2).reshape(128, NBLK * TPB * 128)
        # padmask [128, NBLK]: 1.0 where dst row does not exist (beyond NPC)
        pm = np.zeros((128, NBLK), np.float32)
        for b in range(NBLK):
            nd = min(128, NPC - b * 128)
            if nd < 128:
                pm[nd:, b] = 1.0
        # combined train idx: per 2048-chunk: [a_chunk | b_chunk]
        ca = pair_nodes[0, c * TEC:(c + 1) * TEC]
        cb = pair_nodes[1, c * TEC:(c + 1) * TEC]
        segs = []
        for e0 in range(0, TEC, EC):
            segs.append(ca[e0:e0 + EC])
            segs.append(cb[e0:e0 + EC])
        c_idx = _wrap_idx(np.concatenate(segs))
        cores.append(dict(
            hx_idx=hx_w, padmask=pm, c_idx=c_idx,
            Sb=np.ascontiguousarray(S_blob.astype(ml_dtypes.bfloat16)),
            Stb=np.ascontiguousarray(St_blob.astype(ml_dtypes.bfloat16))))

    # weights (shared across cores)
    g = {k: np.asarray(v).astype(np.float32) for k, v in inputs.items()
         if k not in ("edge_index", "train_edge_id")}
    A1s = np.zeros((F1, H1), np.float32)
    A1d = np.zeros((F1, H1), np.float32)
    for h in range(H1):
        A1s[h * C1:(h + 1) * C1, h] = g["gat1_asrc"][h]
        A1d[h * C1:(h + 1) * C1, h] = g["gat1_adst"][h]
    ga1 = np.concatenate([g["gat1_w"] @ A1s, g["gat1_w"] @ A1d], axis=1)  # [384,16]
    ga2 = np.concatenate([g["gat2_w"] @ g["gat2_asrc"].T,
                          g["gat2_w"] @ g["gat2_adst"].T], axis=1)        # [256,2]
    # fc1 fused into gat1 (z1 has no other consumer); fc5 fused into the
    # gat2 h2/a2 path (z5 kept separately for the residual)
    f64 = np.float64
    w1f = np.concatenate([g["fc1_w"].astype(f64) @ g["gat1_w"].astype(f64),
                          g["fc1_w"].astype(f64) @ ga1.astype(f64)],
                         axis=1).astype(np.float32)              # [384, 400]
    b1f = np.concatenate([g["fc1_b"].astype(f64) @ g["gat1_w"].astype(f64),
                          g["fc1_b"].astype(f64) @ ga1.astype(f64)]
                         ).astype(np.float32)                    # [400]
    w5f = np.concatenate([g["fc5_w"].astype(f64) @ g["gat2_w"].astype(f64),
                          g["fc5_w"].astype(f64) @ ga2.astype(f64)],
                         axis=1).astype(np.float32)              # [384, 258]
    b5f = np.concatenate([g["fc5_b"].astype(f64) @ g["gat2_w"].astype(f64),
                          g["fc5_b"].astype(f64) @ ga2.astype(f64)]
                         ).astype(np.float32)                    # [258]

    shared = dict(
        w1f=_pack_slabs(w1f, 3),
        b1rep=np.tile(b1f[None, :], (128, 1)).astype(np.float32),
        w5f=_pack_slabs(w5f, 3),
        b5rep=np.tile(b5f[None, :], (128, 1)).astype(np.float32),
        w5=_pack_slabs(g["fc5_w"], 3), b5c=_pack_col(g["fc5_b"], 2),
        w2f=_pack_slabs(g["fc2_w"], 2),
        b2rep=np.tile(g["fc2_b"][None, :], (128, 1)).astype(np.float32),
        w4=_pack_slabs(g["fc4_w"].astype(np.float32), 2),
        b4c=g["fc4_b"].reshape(7, 1).astype(np.float32),
        bn1g=_pack_col(g["bn1_g"], 3), bn1b=_pack_col(g["bn1_b"], 3),
        bn2g=_pack_col(g["bn2_g"], 2), bn2b=_pack_col(g["bn2_b"], 2),
        iota=np.tile(np.arange(128, dtype=np.float32), (128, 1)),
        eye=np.eye(128, dtype=np.float32),
        ones_col=np.ones((128, 1), np.float32),
    )

    x = np.asarray(inputs["x"]).astype(np.float32)
    for c in range(NCORES):
        xc = x[c * NPC:(c + 1) * NPC]              # [1250, 384]
        cores[c]["xT"] = _pack_slabs(np.ascontiguousarray(xc.T), 3)
    return dict(TPB=TPB, cores=cores, shared=shared)


def _build_program(TPB):
    # debug bisect: stop building after phase PH (1..6); 6 = full program
    PH = int(os.environ.get("BASS_GAT_PHASES", "6"))
    nc = bacc.Bacc("TRN2", target_bir_lowering=False, debug=False,
                   num_devices=NCORES, num_swdge_queues=4)

    def din(name, shape, dt=F32):
        return nc.dram_tensor(name, list(shape), dt, kind="ExternalInput").ap()

    NIDX = TPB * 128
    D = dict(
        hx_idx=din("hx_idx", [128, NBLK * TPB * 8], I16),
        padmask=din("padmask", [128, NBLK]),
        c_idx=din("c_idx", [128, 2 * TEC // 16], I16),
        xTb=din("xTb", [128, 3, NPC], BF16),
        w1f=din("w1f", [128, 3, 400], BF16), b1rep=din("b1rep", [128, 400]),
        w5f=din("w5f", [128, 3, 258], BF16), b5rep=din("b5rep", [128, 258]),
        w5=din("w5", [128, 3, F2], BF16), b5c=din("b5c", [128, 2]),
        w2f=din("w2f", [128, 2, F2], BF16), b2rep=din("b2rep", [128, F2]),
        w4=din("w4", [128, 2, 7]), b4c=din("b4c", [7, 1]),
        bn1g=din("bn1g", [128, 3]), bn1b=din("bn1b", [128, 3]),
        bn2g=din("bn2g", [128, 2]), bn2b=din("bn2b", [128, 2]),
        iota=din("iota", [128, 128]), eye=din("eye", [128, 128]),
        ones_col=din("ones_col", [128, 1]),
    )
    S_dram = nc.dram_tensor("Sb", [128, NBLK * NIDX], BF16,
                            kind="ExternalInput").ap()
    St_dram = nc.dram_tensor("Stb", [128, NBLK * NIDX], BF16,
                             kind="ExternalInput").ap()
    out_t = nc.dram_tensor("out_t", [7, TEC], F32, kind="ExternalOutput").ap()

    with tile.TileContext(nc) as tc:
        with tc.tile_pool(name="persist", bufs=1) as pp, \
             tc.tile_pool(name="dram", bufs=1, space="DRAM") as dd:
            # ---- persistent SBUF loads ----
            sb = {}
            for k, ap in D.items():
                t = pp.tile(list(ap.shape), ap.dtype, tag=f"in_{k}")
                nc.sync.dma_start(out=t[:], in_=ap)
                sb[k] = t
            # bf16 copies of constants used by bf16 ops
            w4b = pp.tile([128, 2, 7], BF16, tag="w4b")
            nc.vector.tensor_copy(out=w4b[:], in_=sb["w4"][:])
            eye_b = pp.tile([128, 128], BF16, tag="eye_b")
            nc.vector.tensor_copy(out=eye_b[:], in_=sb["eye"][:])

            # ---- DRAM bounces ----
            hx1_b = dd.tile([NPC, ROW1], BF16, tag="hx1b")
            hx1_f = dd.tile([N, ROW1], BF16, tag="hx1f", addr_space="Shared")
            hx2_b = dd.tile([NPC, ROW2], BF16, tag="hx2b")
            hx2_f = dd.tile([N, ROW2], BF16, tag="hx2f", addr_space="Shared")
            ho_b = dd.tile([NPC, F2], BF16, tag="hob")
            ho_f = dd.tile([N, F2], BF16, tag="hof", addr_space="Shared")
            bn1_i = dd.tile([1, 2 * F1], F32, tag="bn1i")
            bn1_o = dd.tile([8, 2 * F1], F32, tag="bn1o", addr_space="Shared")
            bn2_i = dd.tile([1, 2 * F2], F32, tag="bn2i")
            bn2_o = dd.tile([8, 2 * F2], F32, tag="bn2o", addr_space="Shared")

            # persistent activations
            z5T = pp.tile([128, 2, NPC], F32, tag="z5T")
            xgT = pp.tile([128, 3, NBLK * 128], F32, tag="xgT")
            xg2T = pp.tile([128, 2, NBLK * 128], F32, tag="xg2T")
            hmidT = pp.tile([128, 3, NPC], BF16, tag="hmidT")
            hfinT = pp.tile([128, 2, NPC], BF16, tag="hfinT")
            adst1o = pp.tile([128, NBLK, H1], BF16, tag="adst1o")
            adst2o = pp.tile([128, NBLK, 1], BF16, tag="adst2o")
            nc.vector.memset(adst1o[:], 0)
            nc.vector.memset(adst2o[:], 0)

            chunks = [(i, min(i + 512, NPC)) for i in range(0, NPC, 512)]
            rg = [list(range(NCORES))]

            def ag_chunk(src, dst_full, lo, hi):
                nc.gpsimd.collective_compute(
                    "AllGather", AO.bypass,
                    ins=[src[lo:hi].opt()],
                    outs=[dst_full[8 * lo:8 * lo + 8 * (hi - lo)].opt()]
                    if lo == 0 else
                    [dst_full[8 * RSP + 8 * (lo - RSP):8 * RSP + 8 * (hi - RSP)].opt()],
                    replica_groups=rg)

            # ================= Phase 1: fused x @ (fc1.gat1) -> hx1
            with tc.tile_pool(name="d1ps", bufs=1, space="PSUM") as d1ps, \
                 tc.tile_pool(name="d1sb", bufs=1) as d1sb:
                for nt0 in range(0, NPC, 128):
                    nt1 = min(nt0 + 128, NPC)
                    R = nt1 - nt0
                    b = nt0 // 128
                    pall = d1ps.tile([128, 400], F32, tag="pall", bufs=3,
                                     name="pall")
                    for k in range(3):
                        nc.tensor.matmul(out=pall[:R, :],
                                         lhsT=sb["xTb"][:, k, nt0:nt1],
                                         rhs=sb["w1f"][:, k, :],
                                         start=(k == 0), stop=(k == 2))
                    hxt = d1sb.tile([128, ROW1], BF16, tag="hxt", bufs=3,
                                    name="hxt")
                    nc.vector.tensor_tensor(out=hxt[:R, 0:F1 + H1],
                                            in0=pall[:R, 0:F1 + H1],
                                            in1=sb["b1rep"][:R, 0:F1 + H1],
                                            op=AO.add)
                    nc.vector.memset(hxt[:R, F1 + H1:ROW1], 0)
                    nc.sync.dma_start(out=hx1_b[nt0:nt1, :], in_=hxt[:R, :])
                    nc.vector.tensor_tensor(out=adst1o[:R, b, :],
                                            in0=pall[:R, 392:400],
                                            in1=sb["b1rep"][:R, 392:400],
                                            op=AO.add)

            def _early_out():
                nc.sync.dma_start(out=out_t[:, 0:128], in_=sb["iota"][0:7, 0:128])

            if PH >= 1:
                ag_chunk(hx1_b, hx1_f, 0 if not CHUNK_AG else RSP, NPC)

            # ================= sparse GAT phase (shared builder)
            def sparse_phase(layer, table, xgTd, adsto, stats_sb, ga=1):
                ROW = ROW1 if layer == 1 else ROW2
                F = F1 if layer == 1 else F2
                H = H1 if layer == 1 else 1
                NS = 3 if layer == 1 else 2
                RC = F + H if layer == 1 else F + 2
                qsz = [TPB // 4 + (1 if i < TPB % 4 else 0) for i in range(4)]
                with tc.tile_pool(name=f"sp{layer}", bufs=1, space="PSUM") as sp, \
                     tc.tile_pool(name=f"sl{layer}", bufs=1) as sl:
                    psum_sum = sp.tile([1, F], F32, tag="st0")
                    psum_ssq = sp.tile([1, F], F32, tag="st1")
                    pda_sb = sl.tile([128, NBLK, TPB * H], F32, tag="pdasb")

                    # ---- prologue: all pda matmuls (hidden under AllGather) ----
                    for b in range(NBLK):
                        Stsb = sl.tile([128, NIDX], BF16, tag="Stsb", bufs=2,
                                       name="Stsb")
                        nc.sync.dma_start(out=Stsb[:],
                                          in_=St_dram[:, b * NIDX:(b + 1) * NIDX])
                        pda = sp.tile([128, TPB * H], F32, tag="pda", bufs=2,
                                      name="pda")
                        for t in range(TPB):
                            nc.tensor.matmul(
                                out=pda[:, t * H:(t + 1) * H],
                                lhsT=Stsb[:, t * 128:(t + 1) * 128],
                                rhs=adsto[:, b, :], start=True, stop=True)
                        nc.scalar.activation(out=pda_sb[:, b, :], in_=pda[:],
                                             func=AF.Identity)

                    def emit_gather(b):
                        gth = sl.tile([128, TPB, ROW], BF16, tag="gth", bufs=3,
                                      name="gth")
                        i0 = b * TPB * 8
                        o = 0
                        for qi, sz in enumerate(qsz):
                            nc.gpsimd.dma_gather(
                                gth[:, o:o + sz, :], table[:],
                                sb["hx_idx"][:, i0 + o * 8:i0 + (o + sz) * 8],
                                num_idxs=sz * 128, num_idxs_reg=sz * 128,
                                elem_size=ROW, single_packet=True,
                                queue_num=qi)
                            o += sz
                        return gth

                    def emit_sload(b):
                        Ssb = sl.tile([128, NIDX], BF16, tag="Ssb", bufs=3,
                                      name="Ssb")
                        nc.sync.dma_start(out=Ssb[:],
                                          in_=S_dram[:, b * NIDX:(b + 1) * NIDX])
                        return Ssb

                    Smats = {b: emit_sload(b) for b in range(3)}
                    gths = {b: emit_gather(b) for b in range(ga)}
                    fin = {}

                    def emit_finalize(b):
                        pnum, Ssb = fin.pop(b)
                        den_lo = F if layer == 1 else F + 1
                        dent = sl.tile([128, H], F32, tag="dent", bufs=2,
                                       name="dent")
                        nc.vector.tensor_scalar(
                            out=dent[:], in0=pnum[:, den_lo:den_lo + H],
                            scalar1=sb["padmask"][:, b:b + 1], scalar2=None,
                            op0=AO.add)
                        rec = sl.tile([128, H], F32, tag="rec", bufs=2,
                                      name="rec")
                        nc.vector.reciprocal(out=rec[:], in_=dent[:])
                        xgt = sl.tile([128, F], F32, tag="xgt", bufs=2,
                                      name="xgt")
                        nc.vector.tensor_tensor(
                            out=xgt[:].rearrange("p (g c) -> p g c", g=H),
                            in0=pnum[:, 0:F].rearrange("p (g c) -> p g c", g=H),
                            in1=rec[:].to_broadcast([128, H, F // H]),
                            op=AO.mult)
                        sq = sl.tile([128, F], F32, tag="sq", bufs=1, name="sq")
                        nc.scalar.activation(out=sq[:], in_=xgt[:],
                                             func=AF.Square)
                        nc.tensor.matmul(out=psum_sum[:], lhsT=sb["ones_col"][:],
                                         rhs=xgt[:],
                                         start=(b == 0), stop=(b == NBLK - 1))
                        nc.tensor.matmul(out=psum_ssq[:], lhsT=sb["ones_col"][:],
                                         rhs=sq[:],
                                         start=(b == 0), stop=(b == NBLK - 1))
                        for s in range(NS):
                            ptx = sp.tile([128, 128], F32, tag="trS", bufs=2,
                                          name="ptx")
                            nc.tensor.transpose(out=ptx[:],
                                                in_=xgt[:, s * 128:(s + 1) * 128],
                                                identity=sb["eye"][:])
                            nc.scalar.activation(
                                out=xgTd[:, s, b * 128:(b + 1) * 128],
                                in_=ptx[:], func=AF.Identity)

                    for b in range(NBLK):
                        if b + ga < NBLK:
                            gths[b + ga] = emit_gather(b + ga)
                        if b + 3 < NBLK:
                            Smats[b + 3] = emit_sload(b + 3)
                        gth = gths.pop(b)
                        Ssb = Smats.pop(b)
                        # logits for the whole block
                        tl = sl.tile([128, TPB, H], F32, tag="tl", bufs=2)
                        nc.vector.tensor_tensor(
                            out=tl[:], in0=gth[:, :, F:F + H],
                            in1=pda_sb[:, b, :].rearrange(
                                "p (t h) -> p t h", h=H),
                            op=AO.add)
                        tl2 = sl.tile([128, TPB, H], F32, tag="tl2", bufs=2)
                        nc.vector.scalar_tensor_tensor(
                            out=tl2[:], in0=tl[:], scalar=NEG_SLOPE, in1=tl[:],
                            op0=AO.mult, op1=AO.max)
                        pnum = sp.tile([128, RC], F32, tag="num", bufs=2)
                        if layer == 1:
                            ctw = sl.tile([128, TPB, RC], BF16, tag="ctw",
                                          bufs=2)
                            nc.scalar.activation(out=ctw[:, :, F:F + H],
                                                 in_=tl2[:], func=AF.Exp)

                            def emit_ctw(lo, hi, eng):
                                eng.tensor_tensor(
                                    out=ctw[:, lo:hi, 0:F].rearrange(
                                        "p t (g c) -> p t g c", c=C1),
                                    in0=gth[:, lo:hi, 0:F].rearrange(
                                        "p t (g c) -> p t g c", c=C1),
                                    in1=ctw[:, lo:hi, F:F + H].to_broadcast(
                                        [128, hi - lo, H, C1]),
                                    op=AO.mult)

                            def num_rhs(t):
                                return Ssb[:, t * 128:(t + 1) * 128], ctw[:, t, :]
                        else:
                            wt1 = sl.tile([128, TPB, 1], F32, tag="wt1", bufs=2)
                            nc.scalar.activation(out=wt1[:], in_=tl2[:],
                                                 func=AF.Exp)
                            S2 = sl.tile([128, NIDX], BF16, tag="S2", bufs=2)

                            def emit_ctw(lo, hi, eng):
                                eng.tensor_tensor(
                                    out=S2[:, lo * 128:hi * 128].rearrange(
                                        "p (t d) -> p t d", d=128),
                                    in0=Ssb[:, lo * 128:hi * 128].rearrange(
                                        "p (t d) -> p t d", d=128),
                                    in1=wt1[:, lo:hi, :].rearrange(
                                        "p t h -> p (t h)").to_broadcast(
                                        [128, hi - lo, 128]),
                                    op=AO.mult)

                            def num_rhs(t):
                                return S2[:, t * 128:(t + 1) * 128], gth[:, t, 0:RC]

                        for (lo, hi) in ((0, TPB // 2), (TPB // 2, TPB)):
                            emit_ctw(lo, hi, nc.vector)
                            for t in range(lo, hi):
                                lhs, rhs = num_rhs(t)
                                nc.tensor.matmul(out=pnum[:, 0:RC], lhsT=lhs,
                                                 rhs=rhs,
                                                 start=(t == 0),
                                                 stop=(t == TPB - 1))
                        fin[b] = (pnum, Ssb)
                        if b > 0:
                            emit_finalize(b - 1)
                    emit_finalize(NBLK - 1)
                    nc.vector.tensor_copy(out=stats_sb[:, 0:F], in_=psum_sum[:])
                    nc.vector.tensor_copy(out=stats_sb[:, F:2 * F],
                                          in_=psum_ssq[:])

            if PH >= 2:
                stats1 = pp.tile([1, 2 * F1], F32, tag="stats1")
                sparse_phase(1, hx1_f, xgT, adst1o, stats1)
                nc.sync.dma_start(out=bn1_i[:], in_=stats1[:])
                nc.gpsimd.collective_compute("AllGather", AO.bypass,
                                             ins=[bn1_i[:].opt()],
                                             outs=[bn1_o[:].opt()],
                                             replica_groups=rg)

            # ================= Phase: BN1 + residual + fc5 + h2/asrc2/adst2
            def bn_scale_shift(bn_o, F, NS, gcol, bcol, pool):
                g8 = pool.tile([128, 8, 2 * NS], F32, tag="g8")
                nc.sync.dma_start(out=g8[:], in_=bn_o[:, :].rearrange(
                    "a (w p) -> p a w", p=128))
                acc8 = pool.tile([128, 2 * NS], F32, tag="acc8")
                nc.vector.tensor_tensor(out=acc8[:], in0=g8[:, 0, :],
                                        in1=g8[:, 1, :], op=AO.add)
                for a in range(2, 8):
                    nc.vector.tensor_tensor(out=acc8[:], in0=acc8[:],
                                            in1=g8[:, a, :], op=AO.add)
                gsum = acc8[:, 0:NS]
                gssq = acc8[:, NS:2 * NS]
                mu = pool.tile([128, NS], F32, tag="mu")
                nc.vector.tensor_scalar(out=mu[:], in0=gsum, scalar1=1.0 / N,
                                        scalar2=None, op0=AO.mult)
                mu2 = pool.tile([128, NS], F32, tag="mu2")
                nc.scalar.activation(out=mu2[:], in_=mu[:], func=AF.Square)
                var = pool.tile([128, NS], F32, tag="var")
                nc.vector.scalar_tensor_tensor(out=var[:], in0=gssq,
                                               scalar=1.0 / N, in1=mu2[:],
                                               op0=AO.mult, op1=AO.subtract)
                nc.vector.tensor_scalar(out=var[:], in0=var[:], scalar1=BN_EPS,
                                        scalar2=None, op0=AO.add)
                sd = pool.tile([128, NS], F32, tag="sd")
                nc.scalar.activation(out=sd[:], in_=var[:], func=AF.Sqrt)
                rstd = pool.tile([128, NS], F32, tag="rstd")
                nc.vector.reciprocal(out=rstd[:], in_=sd[:])
                scale = pool.tile([128, NS], F32, tag="scale")
                nc.vector.tensor_tensor(out=scale[:], in0=gcol[:], in1=rstd[:],
                                        op=AO.mult)
                shift = pool.tile([128, NS], F32, tag="shift")
                nc.vector.tensor_tensor(out=shift[:], in0=mu[:], in1=scale[:],
                                        op=AO.mult)
                nc.vector.tensor_tensor(out=shift[:], in0=bcol[:], in1=shift[:],
                                        op=AO.subtract)
                return scale, shift

            if PH >= 3:
                with tc.tile_pool(name="bn1sb", bufs=1) as bnp, \
                     tc.tile_pool(name="d2ps", bufs=1, space="PSUM") as d2ps, \
                     tc.tile_pool(name="d2sb", bufs=1) as d2sb:
                    scale1, shift1 = bn_scale_shift(bn1_o, F1, 3, sb["bn1g"],
                                                    sb["bn1b"], bnp)
                    for (c0, c1) in chunks:
                        for s in range(3):
                            tmp = d2sb.tile([128, 512], F32, tag="hmt", bufs=2,
                                            name="tmp")
                            nc.vector.scalar_tensor_tensor(
                                out=tmp[:, 0:c1 - c0], in0=xgT[:, s, c0:c1],
                                scalar=scale1[:, s:s + 1],
                                in1=sb["xTb"][:, s, c0:c1],
                                op0=AO.mult, op1=AO.add)
                            nc.scalar.activation(out=hmidT[:, s, c0:c1],
                                                 in_=tmp[:, 0:c1 - c0],
                                                 func=AF.Relu,
                                                 bias=shift1[:, s:s + 1])
                        for nt0 in range(c0, c1, 128):
                            nt1 = min(nt0 + 128, c1)
                            R = nt1 - nt0
                            b = nt0 // 128
                            p2 = d2ps.tile([128, 258], F32, tag="p2", bufs=3,
                                           name="p2")
                            for k in range(3):
                                nc.tensor.matmul(out=p2[:R, :],
                                                 lhsT=hmidT[:, k, nt0:nt1],
                                                 rhs=sb["w5f"][:, k, :],
                                                 start=(k == 0), stop=(k == 2))
                            hxt = d2sb.tile([128, ROW2], BF16, tag="hxt2",
                                            bufs=3, name="hxt")
                            nc.vector.tensor_tensor(out=hxt[:R, 0:F2 + 1],
                                                    in0=p2[:R, 0:F2 + 1],
                                                    in1=sb["b5rep"][:R, 0:F2 + 1],
                                                    op=AO.add)
                            nc.vector.memset(hxt[:R, F2 + 1:F2 + 2], 1.0)
                            nc.vector.memset(hxt[:R, F2 + 2:ROW2], 0)
                            nc.sync.dma_start(out=hx2_b[nt0:nt1, :],
                                              in_=hxt[:R, :])
                            nc.vector.tensor_tensor(out=adst2o[:R, b, :],
                                                    in0=p2[:R, 257:258],
                                                    in1=sb["b5rep"][:R, 257:258],
                                                    op=AO.add)
                    ag_chunk(hx2_b, hx2_f, 0 if not CHUNK_AG else RSP, NPC)
                    # z5 (residual path) — runs in the AllGather shadow
                    def emit_z5(c0, c1):
                        W = c1 - c0
                        for m in range(2):
                            pz = d2ps.tile([128, 512], F32, tag="pz5", bufs=2,
                                           name="pz5")
                            for k in range(3):
                                nc.tensor.matmul(
                                    out=pz[:, :W],
                                    lhsT=sb["w5"][:, k, m * 128:(m + 1) * 128],
                                    rhs=hmidT[:, k, c0:c1],
                                    start=(k == 0), stop=(k == 2))
                            nc.scalar.activation(out=z5T[:, m, c0:c1],
                                                 in_=pz[:, :W],
                                                 func=AF.Identity,
                                                 bias=sb["b5c"][:, m:m + 1])
                    for (c0, c1) in chunks:
                        emit_z5(c0, c1)

            if PH >= 4:
                stats2 = pp.tile([1, 2 * F2], F32, tag="stats2")
                sparse_phase(2, hx2_f, xg2T, adst2o, stats2, ga=2)
                nc.sync.dma_start(out=bn2_i[:], in_=stats2[:])
                nc.gpsimd.collective_compute("AllGather", AO.bypass,
                                             ins=[bn2_i[:].opt()],
                                             outs=[bn2_o[:].opt()],
                                             replica_groups=rg)

            # ================= Phase: BN2 + residual + fc2 -> hout (bf16)
            if PH >= 5:
                with tc.tile_pool(name="bn2sb", bufs=1) as bnp2, \
                     tc.tile_pool(name="d3ps", bufs=1, space="PSUM") as d3ps, \
                     tc.tile_pool(name="d3sb", bufs=1) as d3sb:
                    scale2, shift2 = bn_scale_shift(bn2_o, F2, 2, sb["bn2g"],
                                                    sb["bn2b"], bnp2)
                    for (c0, c1) in chunks:
                        for s in range(2):
                            tmp = d3sb.tile([128, 512], F32, tag="hft", bufs=2,
                                            name="tmp")
                            nc.vector.scalar_tensor_tensor(
                                out=tmp[:, 0:c1 - c0], in0=xg2T[:, s, c0:c1],
                                scalar=scale2[:, s:s + 1], in1=z5T[:, s, c0:c1],
                                op0=AO.mult, op1=AO.add)
                            nc.scalar.activation(out=hfinT[:, s, c0:c1],
                                                 in_=tmp[:, 0:c1 - c0],
                                                 func=AF.Relu,
                                                 bias=shift2[:, s:s + 1])
                        for nt0 in range(c0, c1, 128):
                            nt1 = min(nt0 + 128, c1)
                            R = nt1 - nt0
                            po = d3ps.tile([128, F2], F32, tag="po", bufs=2,
                                           name="po")
                            for k in range(2):
                                nc.tensor.matmul(out=po[:R, :],
                                                 lhsT=hfinT[:, k, nt0:nt1],
                                                 rhs=sb["w2f"][:, k, :],
                                                 start=(k == 0), stop=(k == 1))
                            hob = d3sb.tile([128, F2], BF16, tag="hob", bufs=2,
                                            name="hob")
                            nc.vector.tensor_tensor(out=hob[:R, :],
                                                    in0=po[:R, :],
                                                    in1=sb["b2rep"][:R, :],
                                                    op=AO.add)
                            nc.sync.dma_start(out=ho_b[nt0:nt1, :],
                                              in_=hob[:R, :])
                ag_chunk(ho_b, ho_f, 0 if not CHUNK_AG else RSP, NPC)

            # ================= Phase: train-edge head
            if PH >= 6:
                with tc.tile_pool(name="tps", bufs=1, space="PSUM") as tps, \
                     tc.tile_pool(name="tsb", bufs=1) as tsb:
                    def emit_hgather(ec):
                        gab = tsb.tile([128, 2 * EC // 128, F2], BF16,
                                       tag="gab", bufs=3, name="gab")
                        ic0 = ec * (2 * EC // 16)
                        for j in range(2):
                            nc.gpsimd.dma_gather(
                                gab[:, 8 * j:8 * (j + 1), :], ho_f[:],
                                sb["c_idx"][:, ic0 + 64 * j:ic0 + 64 * (j + 1)],
                                num_idxs=1024, num_idxs_reg=1024, elem_size=F2,
                                single_packet=True,
                                queue_num=(2 * ec + j) % 4)
                        return gab

                    NEC = TEC // EC
                    for ec in range(NEC):
                        gab = emit_hgather(ec)
                        prn = tsb.tile([128, EC // 128, F2], BF16, tag="prn",
                                       bufs=2)
                        EH = EC // 256
                        for ph in range(2):
                            nc.vector.tensor_tensor(
                                out=prn[:, ph * EH:(ph + 1) * EH, :],
                                in0=gab[:, ph * EH:(ph + 1) * EH, :],
                                in1=gab[:, EC // 128 + ph * EH:
                                        EC // 128 + (ph + 1) * EH, :],
                                op=AO.mult)
                        prT = tsb.tile([128, 2, EC], BF16, tag="prT", bufs=2)
                        for et in range(EC // 128):
                            for s in range(2):
                                ptx = tps.tile([128, 128], BF16, tag="ptt",
                                               bufs=2)
                                nc.tensor.transpose(
                                    out=ptx[:],
                                    in_=prn[:, et, s * 128:(s + 1) * 128],
                                    identity=eye_b[:])
                                if et % 3 != 2:
                                    nc.vector.tensor_copy(
                                        out=prT[:, s, et * 128:(et + 1) * 128],
                                        in_=ptx[:])
                                else:
                                    nc.scalar.activation(
                                        out=prT[:, s, et * 128:(et + 1) * 128],
                                        in_=ptx[:], func=AF.Identity)
                        ot = tsb.tile([7, EC], F32, tag="ot", bufs=2)
                        for et in range(EC // 512):
                            po = tps.tile([7, 512], F32, tag="pot", bufs=2)
                            for k in range(2):
                                nc.tensor.matmul(
                                    out=po[:, :], lhsT=w4b[:, k, :],
                                    rhs=prT[:, k, et * 512:(et + 1) * 512],
                                    start=(k == 0), stop=(k == 1))
                            nc.scalar.activation(out=ot[:, et * 512:(et + 1) * 512],
                                                 in_=po[:, :], func=AF.Identity,
                                                 bias=sb["b4c"][:, 0:1])
                        nc.sync.dma_start(out=out_t[:, ec * EC:(ec + 1) * EC],
                                          in_=ot[:, :])
            if PH < 6:
                _early_out()

    nc.compile()
    return nc


def _make_in_maps(prep):
    shared = prep["shared"]
    maps = []
    for c in range(NCORES):
        m = {}
        for k in ("b1rep", "b5rep", "b5c", "b2rep", "w4", "b4c",
                  "bn1g", "bn1b", "bn2g", "bn2b",
                  "iota", "eye", "ones_col"):
            m[k] = np.ascontiguousarray(shared[k].astype(np.float32))
        for k in ("w1f", "w5f", "w5", "w2f"):
            m[k] = np.ascontiguousarray(shared[k].astype(ml_dtypes.bfloat16))
        cd = prep["cores"][c]
        m["xTb"] = np.ascontiguousarray(cd["xT"].astype(ml_dtypes.bfloat16))
        m["hx_idx"] = np.ascontiguousarray(cd["hx_idx"])
        m["padmask"] = np.ascontiguousarray(cd["padmask"])
        m["c_idx"] = np.ascontiguousarray(cd["c_idx"])
        m["Sb"] = cd["Sb"]
        m["Stb"] = cd["Stb"]
        maps.append(m)
    return maps


def _ensure_ntff_hook():
    """Register the NTFF profile hook (missing antenv.axon_hooks shim)."""
    import sys, types
    if "antenv.axon_hooks" not in sys.modules:
        mod = types.ModuleType("antenv.axon_hooks")
        _h = [None]
        mod.set_axon_ntff_profile_hook = lambda h: _h.__setitem__(0, h)
        mod.get_axon_ntff_profile_hook = lambda: _h[0]
        sys.modules["antenv.axon_hooks"] = mod
        import antenv
        antenv.axon_hooks = mod
    import antenv.axon_hooks as ah
    if ah.get_axon_ntff_profile_hook() is None:
        try:
            from trn_agent_boot.trn_boot import _ntff_profile_via_ctypes
            ah.set_axon_ntff_profile_hook(
                _ntff_profile_via_ctypes("/opt/axon/libaxon_pjrt.so"))
        except Exception:
            pass


def kernel(**inputs):
    global last_exec_time_ns
    prep = _prepare(inputs)
    TPB = prep["TPB"]
    if TPB not in _PROG_CACHE:
        _PROG_CACHE[TPB] = _build_program(TPB)
    nc = _PROG_CACHE[TPB]
    in_maps = _make_in_maps(prep)
    trace = os.environ.get("BASS_GAT_TRACE", "0") == "1"
    if trace:
        _ensure_ntff_hook()
    res = run_bass_kernel_spmd(nc, in_maps, core_ids=list(range(NCORES)),
                               trace=trace)
    if trace:
        last_exec_time_ns = res.exec_time_ns
    out = np.concatenate(
        [res.results[c]["out_t"].T for c in range(NCORES)], axis=0)
    return out.astype(np.float32)



# revision 13
# speedup vs baseline: 1.1145x; 1.1145x over previous
"""Trainium2 Bass kernel for nn_GAT_Top (2-layer GAT + FC/BN + DistMult edge head).

Self-contained: takes FULL inputs, shards across 8 NeuronCores internally
(dst-node ownership for the sparse phases, node-parallel dense phases, halo
exchange via AllGather of node-feature tables), returns the FULL output.

v4: S scatter matrices built on-chip (1 DVE is_equal per block) instead of
streamed; St streamed once and persisted in SBUF across both sparse layers;
train-edge head uses transposed gathers (no PE transposes / copies) with
descriptor-gen hoisted into the BN2/fc2/AG3 window via prepare_only+trigger.
"""
import os
import numpy as np
import ml_dtypes

import concourse.bass as bass
import concourse.bacc as bacc
import concourse.tile as tile
from concourse import mybir
from concourse.bass_utils import run_bass_kernel_spmd

F32 = mybir.dt.float32
BF16 = mybir.dt.bfloat16
I16 = mybir.dt.int16
AO = mybir.AluOpType
AF = mybir.ActivationFunctionType

# problem constants (hardcoded per harness contract)
N, E, TE = 10000, 160000, 65536
NCORES = 8
NPC = N // NCORES            # 1250 nodes per core
NBLK = 10                    # dst blocks of 128 per core (last has 98)
TEC = TE // NCORES           # 8192 train edges per core
F1, F2 = 384, 256
H1 = 8
C1 = 48
ROW1 = 512                   # hx1 row bf16: h(384) | asrc(8) | pad
ROW2 = 384                   # hx2 row bf16: h2(256) | asrc2(1) | one(1) | pad
NEG_SLOPE = 0.2
BN_EPS = 1e-5
PADLOC = 999.0
EC = 1024                    # train sub-chunk
RSP = 1250                   # AllGather chunk split disabled (single-writer rule)
CHUNK_AG = False

last_exec_time_ns = None
_PROG_CACHE = {}


def _wrap_idx(idx):
    """int idx [n] (n%16==0) -> int16 [128, n//16]: i at [i%16, i//16], replicated
    across the 8 groups of 16 partitions (gpsimd cores)."""
    n = idx.shape[0]
    w = np.asarray(idx, np.int16).reshape(n // 16, 16).T  # [16, n//16]
    return np.tile(w, (8, 1))


def _pack_slabs(w, nslab):
    """[K, M] -> [128, nslab, M] with [p, s, m] = w[s*128+p, m]."""
    K, M = w.shape
    out = np.zeros((128, nslab, M), w.dtype)
    for s in range(nslab):
        k0, k1 = s * 128, min((s + 1) * 128, K)
        out[: k1 - k0, s, :] = w[k0:k1, :]
    return out


def _pack_col(v, nslab):
    """[K] -> [128, nslab] with [p, s] = v[s*128+p]."""
    K = v.shape[0]
    out = np.zeros((128, nslab), v.dtype)
    for s in range(nslab):
        k0, k1 = s * 128, min((s + 1) * 128, K)
        out[: k1 - k0, s] = v[k0:k1]
    return out


def _remap_rows(n):
    """Global node id -> row in the AG-chunk-major gather tables.

    Tables are built by two chunked AllGathers: rows [0, 8*RSP) hold each
    core's local rows [0, RSP), rows [8*RSP, N) hold local rows [RSP, NPC).
    """
    c = n // NPC
    r = n % NPC
    return np.where(r < RSP, c * RSP + r,
                    8 * RSP + c * (NPC - RSP) + (r - RSP))


def _prepare(inputs):
    """Host-side preprocessing: edge partitioning/sorting/padding + weight packing."""
    ei = np.asarray(inputs["edge_index"]).astype(np.int64)
    loops = np.arange(N, dtype=np.int64)
    src = np.concatenate([ei[0], loops])
    dst = np.concatenate([ei[1], loops])

    per_core = []
    tpb_max = 1
    for c in range(NCORES):
        sel = (dst // NPC) == c
        s_c, d_c = src[sel], dst[sel]
        order = np.argsort(d_c, kind="stable")
        s_c, d_c = s_c[order], d_c[order]
        blocks = []
        base = c * NPC
        for b in range(NBLK):
            lo, hi = base + b * 128, min(base + (b + 1) * 128, base + NPC)
            m = (d_c >= lo) & (d_c < hi)
            bs, bd = s_c[m], d_c[m] - lo
            blocks.append((bs, bd))
            tpb_max = max(tpb_max, (len(bs) + 127) // 128)
        per_core.append(blocks)
    TPB = tpb_max + (tpb_max % 2)  # even, so gathers split in halves

    # per-core index blobs
    cores = []
    tid = np.asarray(inputs["train_edge_id"]).astype(np.int64)
    pair_nodes = _remap_rows(ei[:, tid])  # [2, TE] rows in ho table
    d_iota = np.arange(128)
    for c in range(NCORES):
        hx_idx = np.zeros((NBLK, TPB * 128), np.int64)
        dstloc = np.full((NBLK, TPB * 128), PADLOC, np.float32)
        for b, (bs, bd) in enumerate(per_core[c]):
            n = len(bs)
            hx_idx[b, :n] = _remap_rows(bs)
            dstloc[b, :n] = bd.astype(np.float32)
        hx_w = np.concatenate([_wrap_idx(hx_idx[b]) for b in range(NBLK)], axis=1)
        # St scatter matrix [dst part, (b,t,edge)] (streamed once, persisted)
        dl = dstloc.reshape(NBLK, TPB, 128)                  # [b, t, e]
        S4 = (dl[:, :, :, None] == d_iota[None, None, None, :])   # [b,t,e,d]
        St_blob = S4.transpose(3, 0, 1, 2).reshape(128, NBLK * TPB * 128)
        # dstloc columns [e part, (b,t)] for on-chip S builds
        dloc_col = np.ascontiguousarray(
            dl.transpose(2, 0, 1).reshape(128, NBLK * TPB))
        # padmask [128, NBLK]: 1.0 where dst row does not exist (beyond NPC)
        pm = np.zeros((128, NBLK), np.float32)
        for b in range(NBLK):
            nd = min(128, NPC - b * 128)
            if nd < 128:
                pm[nd:, b] = 1.0
        # combined train idx: per 2048-chunk: [a_chunk | b_chunk]
        ca = pair_nodes[0, c * TEC:(c + 1) * TEC]
        cb = pair_nodes[1, c * TEC:(c + 1) * TEC]
        segs = []
        for e0 in range(0, TEC, EC):
            segs.append(ca[e0:e0 + EC])
            segs.append(cb[e0:e0 + EC])
        c_idx = _wrap_idx(np.concatenate(segs))
        cores.append(dict(
            hx_idx=hx_w, padmask=pm, c_idx=c_idx,
            dloc_col=dloc_col.astype(ml_dtypes.bfloat16),
            Stb=np.ascontiguousarray(St_blob.astype(ml_dtypes.bfloat16))))

    # weights (shared across cores)
    g = {k: np.asarray(v).astype(np.float32) for k, v in inputs.items()
         if k not in ("edge_index", "train_edge_id")}
    A1s = np.zeros((F1, H1), np.float32)
    A1d = np.zeros((F1, H1), np.float32)
    for h in range(H1):
        A1s[h * C1:(h + 1) * C1, h] = g["gat1_asrc"][h]
        A1d[h * C1:(h + 1) * C1, h] = g["gat1_adst"][h]
    ga1 = np.concatenate([g["gat1_w"] @ A1s, g["gat1_w"] @ A1d], axis=1)  # [384,16]
    ga2 = np.concatenate([g["gat2_w"] @ g["gat2_asrc"].T,
                          g["gat2_w"] @ g["gat2_adst"].T], axis=1)        # [256,2]
    # fc1 fused into gat1 (z1 has no other consumer); fc5 fused into the
    # gat2 h2/a2 path (z5 kept separately for the residual)
    f64 = np.float64
    w1f = np.concatenate([g["fc1_w"].astype(f64) @ g["gat1_w"].astype(f64),
                          g["fc1_w"].astype(f64) @ ga1.astype(f64)],
                         axis=1).astype(np.float32)              # [384, 400]
    b1f = np.concatenate([g["fc1_b"].astype(f64) @ g["gat1_w"].astype(f64),
                          g["fc1_b"].astype(f64) @ ga1.astype(f64)]
                         ).astype(np.float32)                    # [400]
    w5f = np.concatenate([g["fc5_w"].astype(f64) @ g["gat2_w"].astype(f64),
                          g["fc5_w"].astype(f64) @ ga2.astype(f64)],
                         axis=1).astype(np.float32)              # [384, 258]
    b5f = np.concatenate([g["fc5_b"].astype(f64) @ g["gat2_w"].astype(f64),
                          g["fc5_b"].astype(f64) @ ga2.astype(f64)]
                         ).astype(np.float32)                    # [258]

    iota_tpb = np.tile(np.arange(128, dtype=np.float32), TPB)[None, :].repeat(
        128, axis=0)                                             # [128, TPB*128]
    shared = dict(
        w1f=_pack_slabs(w1f, 3),
        b1rep=np.tile(b1f[None, :], (128, 1)).astype(np.float32),
        w5f=_pack_slabs(w5f, 3),
        b5rep=np.tile(b5f[None, :], (128, 1)).astype(np.float32),
        w5=_pack_slabs(g["fc5_w"], 3), b5c=_pack_col(g["fc5_b"], 2),
        w2f=_pack_slabs(g["fc2_w"], 2),
        b2rep=np.tile(g["fc2_b"][None, :], (128, 1)).astype(np.float32),
        w4=_pack_slabs(g["fc4_w"].astype(np.float32), 2),
        b4c=g["fc4_b"].reshape(7, 1).astype(np.float32),
        bn1g=_pack_col(g["bn1_g"], 3), bn1b=_pack_col(g["bn1_b"], 3),
        bn2g=_pack_col(g["bn2_g"], 2), bn2b=_pack_col(g["bn2_b"], 2),
        iota_tpb=iota_tpb.astype(ml_dtypes.bfloat16),
        eye=np.eye(128, dtype=np.float32),
        ones_col=np.ones((128, 1), np.float32),
    )

    x = np.asarray(inputs["x"]).astype(np.float32)
    for c in range(NCORES):
        xc = x[c * NPC:(c + 1) * NPC]              # [1250, 384]
        cores[c]["xT"] = _pack_slabs(np.ascontiguousarray(xc.T), 3)
    return dict(TPB=TPB, cores=cores, shared=shared)


def _build_program(TPB):
    # debug bisect: stop building after phase PH (1..6); 6 = full program
    PH = int(os.environ.get("BASS_GAT_PHASES", "6"))
    nc = bacc.Bacc("TRN2", target_bir_lowering=False, debug=False,
                   num_devices=NCORES, num_swdge_queues=4)

    def din(name, shape, dt=F32):
        return nc.dram_tensor(name, list(shape), dt, kind="ExternalInput").ap()

    NIDX = TPB * 128
    D = dict(
        hx_idx=din("hx_idx", [128, NBLK * TPB * 8], I16),
        padmask=din("padmask", [128, NBLK]),
        c_idx=din("c_idx", [128, 2 * TEC // 16], I16),
        xTb=din("xTb", [128, 3, NPC], BF16),
        w1f=din("w1f", [128, 3, 400], BF16), b1rep=din("b1rep", [128, 400]),
        w5f=din("w5f", [128, 3, 258], BF16), b5rep=din("b5rep", [128, 258]),
        w5=din("w5", [128, 3, F2], BF16), b5c=din("b5c", [128, 2]),
        w2f=din("w2f", [128, 2, F2], BF16), b2rep=din("b2rep", [128, F2]),
        w4=din("w4", [128, 2, 7]), b4c=din("b4c", [7, 1]),
        bn1g=din("bn1g", [128, 3]), bn1b=din("bn1b", [128, 3]),
        bn2g=din("bn2g", [128, 2]), bn2b=din("bn2b", [128, 2]),
        iota_tpb=din("iota_tpb", [128, NIDX], BF16),
        dloc_col=din("dloc_col", [128, NBLK * TPB], BF16),
        eye=din("eye", [128, 128]),
        ones_col=din("ones_col", [128, 1]),
        # keep St last so phase-1 weights/activations win the DMA race
        Stb=din("Stb", [128, NBLK * NIDX], BF16),
    )
    out_t = nc.dram_tensor("out_t", [7, TEC], F32, kind="ExternalOutput").ap()

    with tile.TileContext(nc) as tc:
        with tc.tile_pool(name="persist", bufs=1) as pp, \
             tc.tile_pool(name="dram", bufs=1, space="DRAM") as dd:
            # ---- persistent SBUF loads ----
            sb = {}
            for k, ap in D.items():
                t = pp.tile(list(ap.shape), ap.dtype, tag=f"in_{k}")
                nc.sync.dma_start(out=t[:], in_=ap)
                sb[k] = t
            # bf16 copies of constants used by bf16 ops
            w4b = pp.tile([128, 2, 7], BF16, tag="w4b")
            nc.vector.tensor_copy(out=w4b[:], in_=sb["w4"][:])
            eye_b = pp.tile([128, 128], BF16, tag="eye_b")
            nc.vector.tensor_copy(out=eye_b[:], in_=sb["eye"][:])

            # ---- DRAM bounces ----
            hx1_b = dd.tile([NPC, ROW1], BF16, tag="hx1b")
            hx1_f = dd.tile([N, ROW1], BF16, tag="hx1f", addr_space="Shared")
            hx2_b = dd.tile([NPC, ROW2], BF16, tag="hx2b")
            hx2_f = dd.tile([N, ROW2], BF16, tag="hx2f", addr_space="Shared")
            ho_b = dd.tile([NPC, F2], BF16, tag="hob")
            ho_f = dd.tile([N, F2], BF16, tag="hof", addr_space="Shared")
            bn1_i = dd.tile([1, 2 * F1], F32, tag="bn1i")
            bn1_o = dd.tile([8, 2 * F1], F32, tag="bn1o", addr_space="Shared")
            bn2_i = dd.tile([1, 2 * F2], F32, tag="bn2i")
            bn2_o = dd.tile([8, 2 * F2], F32, tag="bn2o", addr_space="Shared")

            # persistent activations
            z5T = pp.tile([128, 2, NPC], F32, tag="z5T")
            xgT = pp.tile([128, 3, NBLK * 128], BF16, tag="xgT")
            xg2T = pp.tile([128, 2, NBLK * 128], BF16, tag="xg2T")
            hmidT = pp.tile([128, 3, NPC], BF16, tag="hmidT")
            hfinT = pp.tile([128, 2, NPC], BF16, tag="hfinT")
            adst1o = pp.tile([128, NBLK, H1], BF16, tag="adst1o")
            adst2o = pp.tile([128, NBLK, 1], BF16, tag="adst2o")
            nc.vector.memset(adst1o[:], 0)
            nc.vector.memset(adst2o[:], 0)

            chunks = [(i, min(i + 512, NPC)) for i in range(0, NPC, 512)]
            rg = [list(range(NCORES))]

            def ag_chunk(src, dst_full, lo, hi):
                nc.gpsimd.collective_compute(
                    "AllGather", AO.bypass,
                    ins=[src[lo:hi].opt()],
                    outs=[dst_full[8 * lo:8 * lo + 8 * (hi - lo)].opt()]
                    if lo == 0 else
                    [dst_full[8 * RSP + 8 * (lo - RSP):8 * RSP + 8 * (hi - RSP)].opt()],
                    replica_groups=rg)

            # ================= Phase 1: fused x @ (fc1.gat1) -> hx1
            with tc.tile_pool(name="d1ps", bufs=1, space="PSUM") as d1ps, \
                 tc.tile_pool(name="d1sb", bufs=1) as d1sb:
                for nt0 in range(0, NPC, 128):
                    nt1 = min(nt0 + 128, NPC)
                    R = nt1 - nt0
                    b = nt0 // 128
                    pall = d1ps.tile([128, 400], F32, tag="pall", bufs=3,
                                     name="pall")
                    for k in range(3):
                        nc.tensor.matmul(out=pall[:R, :],
                                         lhsT=sb["xTb"][:, k, nt0:nt1],
                                         rhs=sb["w1f"][:, k, :],
                                         start=(k == 0), stop=(k == 2))
                    hxt = d1sb.tile([128, ROW1], BF16, tag="hxt", bufs=3,
                                    name="hxt")
                    nc.vector.tensor_tensor(out=hxt[:R, 0:F1 + H1],
                                            in0=pall[:R, 0:F1 + H1],
                                            in1=sb["b1rep"][:R, 0:F1 + H1],
                                            op=AO.add)
                    nc.vector.memset(hxt[:R, F1 + H1:ROW1], 0)
                    nc.sync.dma_start(out=hx1_b[nt0:nt1, :], in_=hxt[:R, :])
                    nc.vector.tensor_tensor(out=adst1o[:R, b, :],
                                            in0=pall[:R, 392:400],
                                            in1=sb["b1rep"][:R, 392:400],
                                            op=AO.add)

            def _early_out():
                nc.sync.dma_start(out=out_t[:, 0:128], in_=sb["eye"][0:7, 0:128])

            if PH >= 1:
                ag_chunk(hx1_b, hx1_f, 0 if not CHUNK_AG else RSP, NPC)

            # ================= sparse GAT phase (shared builder)
            def sparse_phase(layer, table, xgTd, adsto, stats_sb, ga=1):
                ROW = ROW1 if layer == 1 else ROW2
                F = F1 if layer == 1 else F2
                H = H1 if layer == 1 else 1
                NS = 3 if layer == 1 else 2
                RC = F + H if layer == 1 else F + 2
                qsz = [TPB // 4 + (1 if i < TPB % 4 else 0) for i in range(4)]
                with tc.tile_pool(name=f"sp{layer}", bufs=1, space="PSUM") as sp, \
                     tc.tile_pool(name=f"sl{layer}", bufs=1) as sl:
                    psum_sum = sp.tile([1, F], F32, tag="st0")
                    psum_ssq = sp.tile([1, F], F32, tag="st1")
                    pda_sb = sl.tile([128, NBLK, TPB * H], F32, tag="pdasb")

                    # ---- prologue: all pda matmuls (St persistent in SBUF) ----
                    for b in range(NBLK):
                        pda = sp.tile([128, TPB * H], F32, tag="pda", bufs=2,
                                      name="pda")
                        for t in range(TPB):
                            nc.tensor.matmul(
                                out=pda[:, t * H:(t + 1) * H],
                                lhsT=sb["Stb"][:, b * NIDX + t * 128:
                                               b * NIDX + (t + 1) * 128],
                                rhs=adsto[:, b, :], start=True, stop=True)
                        nc.scalar.activation(out=pda_sb[:, b, :], in_=pda[:],
                                             func=AF.Identity)

                    def emit_gather(b):
                        gth = sl.tile([128, TPB, ROW], BF16, tag="gth",
                                      bufs=(2 if layer == 1 else 3), name="gth")
                        i0 = b * TPB * 8
                        o = 0
                        for qi, sz in enumerate(qsz):
                            nc.gpsimd.dma_gather(
                                gth[:, o:o + sz, :], table[:],
                                sb["hx_idx"][:, i0 + o * 8:i0 + (o + sz) * 8],
                                num_idxs=sz * 128, num_idxs_reg=sz * 128,
                                elem_size=ROW, single_packet=True,
                                queue_num=qi)
                            o += sz
                        return gth

                    def emit_sbuild(b):
                        # S[e, (t,d)] = (d == dstloc[b, t, e]) : one DVE op
                        Ssb = sl.tile([128, NIDX], BF16, tag="Ssb", bufs=2,
                                      name="Ssb")
                        nc.vector.tensor_tensor(
                            out=Ssb[:].rearrange("p (t d) -> p t d", d=128),
                            in0=sb["iota_tpb"][:].rearrange(
                                "p (t d) -> p t d", d=128),
                            in1=sb["dloc_col"][:, b * TPB:(b + 1) * TPB]
                                .to_broadcast([128, TPB, 128]),
                            op=AO.is_equal)
                        return Ssb

                    Smats = {b: emit_sbuild(b) for b in range(1)}
                    gths = {b: emit_gather(b) for b in range(ga)}
                    fin = {}

                    def emit_finalize(b):
                        pnum = fin.pop(b)
                        den_lo = F if layer == 1 else F + 1
                        dent = sl.tile([128, H], F32, tag="dent", bufs=2,
                                       name="dent")
                        nc.vector.tensor_scalar(
                            out=dent[:], in0=pnum[:, den_lo:den_lo + H],
                            scalar1=sb["padmask"][:, b:b + 1], scalar2=None,
                            op0=AO.add)
                        rec = sl.tile([128, H], F32, tag="rec", bufs=2,
                                      name="rec")
                        nc.vector.reciprocal(out=rec[:], in_=dent[:])
                        xgt = sl.tile([128, F], F32, tag="xgt", bufs=2,
                                      name="xgt")
                        nc.vector.tensor_tensor(
                            out=xgt[:].rearrange("p (g c) -> p g c", g=H),
                            in0=pnum[:, 0:F].rearrange("p (g c) -> p g c", g=H),
                            in1=rec[:].to_broadcast([128, H, F // H]),
                            op=AO.mult)
                        sq = sl.tile([128, F], F32, tag="sq", bufs=1, name="sq")
                        nc.scalar.activation(out=sq[:], in_=xgt[:],
                                             func=AF.Square)
                        nc.tensor.matmul(out=psum_sum[:], lhsT=sb["ones_col"][:],
                                         rhs=xgt[:],
                                         start=(b == 0), stop=(b == NBLK - 1))
                        nc.tensor.matmul(out=psum_ssq[:], lhsT=sb["ones_col"][:],
                                         rhs=sq[:],
                                         start=(b == 0), stop=(b == NBLK - 1))
                        for s in range(NS):
                            ptx = sp.tile([128, 128], F32, tag="trS", bufs=2,
                                          name="ptx")
                            nc.tensor.transpose(out=ptx[:],
                                                in_=xgt[:, s * 128:(s + 1) * 128],
                                                identity=sb["eye"][:])
                            nc.scalar.activation(
                                out=xgTd[:, s, b * 128:(b + 1) * 128],
                                in_=ptx[:], func=AF.Identity)

                    for b in range(NBLK):
                        if b + ga < NBLK:
                            gths[b + ga] = emit_gather(b + ga)
                        if b + 1 < NBLK:
                            Smats[b + 1] = emit_sbuild(b + 1)
                        gth = gths.pop(b)
                        Ssb = Smats.pop(b)
                        # logits for the whole block
                        tl = sl.tile([128, TPB, H], F32, tag="tl", bufs=2)
                        nc.vector.tensor_tensor(
                            out=tl[:], in0=gth[:, :, F:F + H],
                            in1=pda_sb[:, b, :].rearrange(
                                "p (t h) -> p t h", h=H),
                            op=AO.add)
                        tl2 = sl.tile([128, TPB, H], F32, tag="tl2", bufs=2)
                        nc.vector.scalar_tensor_tensor(
                            out=tl2[:], in0=tl[:], scalar=NEG_SLOPE, in1=tl[:],
                            op0=AO.mult, op1=AO.max)
                        pnum = sp.tile([128, RC], F32, tag="num", bufs=2)
                        if layer == 1:
                            ctw = sl.tile([128, TPB, RC], BF16, tag="ctw",
                                          bufs=2)
                            nc.scalar.activation(out=ctw[:, :, F:F + H],
                                                 in_=tl2[:], func=AF.Exp)

                            def emit_ctw(lo, hi, eng):
                                eng.tensor_tensor(
                                    out=ctw[:, lo:hi, 0:F].rearrange(
                                        "p t (g c) -> p t g c", c=C1),
                                    in0=gth[:, lo:hi, 0:F].rearrange(
                                        "p t (g c) -> p t g c", c=C1),
                                    in1=ctw[:, lo:hi, F:F + H].to_broadcast(
                                        [128, hi - lo, H, C1]),
                                    op=AO.mult)

                            def num_rhs(t):
                                return Ssb[:, t * 128:(t + 1) * 128], ctw[:, t, :]
                        else:
                            wt1 = sl.tile([128, TPB, 1], F32, tag="wt1", bufs=2)
                            nc.scalar.activation(out=wt1[:], in_=tl2[:],
                                                 func=AF.Exp)
                            S2 = sl.tile([128, NIDX], BF16, tag="S2", bufs=2)

                            def emit_ctw(lo, hi, eng):
                                eng.tensor_tensor(
                                    out=S2[:, lo * 128:hi * 128].rearrange(
                                        "p (t d) -> p t d", d=128),
                                    in0=Ssb[:, lo * 128:hi * 128].rearrange(
                                        "p (t d) -> p t d", d=128),
                                    in1=wt1[:, lo:hi, :].rearrange(
                                        "p t h -> p (t h)").to_broadcast(
                                        [128, hi - lo, 128]),
                                    op=AO.mult)

                            def num_rhs(t):
                                return S2[:, t * 128:(t + 1) * 128], gth[:, t, 0:RC]

                        for (lo, hi) in ((0, TPB // 2), (TPB // 2, TPB)):
                            emit_ctw(lo, hi, nc.vector)
                            for t in range(lo, hi):
                                lhs, rhs = num_rhs(t)
                                nc.tensor.matmul(out=pnum[:, 0:RC], lhsT=lhs,
                                                 rhs=rhs,
                                                 start=(t == 0),
                                                 stop=(t == TPB - 1))
                        fin[b] = pnum
                        if b > 0:
                            emit_finalize(b - 1)
                    emit_finalize(NBLK - 1)
                    nc.vector.tensor_copy(out=stats_sb[:, 0:F], in_=psum_sum[:])
                    nc.vector.tensor_copy(out=stats_sb[:, F:2 * F],
                                          in_=psum_ssq[:])

            if PH >= 2:
                stats1 = pp.tile([1, 2 * F1], F32, tag="stats1")
                sparse_phase(1, hx1_f, xgT, adst1o, stats1)
                nc.sync.dma_start(out=bn1_i[:], in_=stats1[:])
                nc.gpsimd.collective_compute("AllGather", AO.bypass,
                                             ins=[bn1_i[:].opt()],
                                             outs=[bn1_o[:].opt()],
                                             replica_groups=rg)

            # ================= Phase: BN1 + residual + fc5 + h2/asrc2/adst2
            def bn_scale_shift(bn_o, F, NS, gcol, bcol, pool):
                g8 = pool.tile([128, 8, 2 * NS], F32, tag="g8")
                nc.sync.dma_start(out=g8[:], in_=bn_o[:, :].rearrange(
                    "a (w p) -> p a w", p=128))
                acc8 = pool.tile([128, 2 * NS], F32, tag="acc8")
                nc.vector.tensor_tensor(out=acc8[:], in0=g8[:, 0, :],
                                        in1=g8[:, 1, :], op=AO.add)
                for a in range(2, 8):
                    nc.vector.tensor_tensor(out=acc8[:], in0=acc8[:],
                                            in1=g8[:, a, :], op=AO.add)
                gsum = acc8[:, 0:NS]
                gssq = acc8[:, NS:2 * NS]
                mu = pool.tile([128, NS], F32, tag="mu")
                nc.vector.tensor_scalar(out=mu[:], in0=gsum, scalar1=1.0 / N,
                                        scalar2=None, op0=AO.mult)
                mu2 = pool.tile([128, NS], F32, tag="mu2")
                nc.scalar.activation(out=mu2[:], in_=mu[:], func=AF.Square)
                var = pool.tile([128, NS], F32, tag="var")
                nc.vector.scalar_tensor_tensor(out=var[:], in0=gssq,
                                               scalar=1.0 / N, in1=mu2[:],
                                               op0=AO.mult, op1=AO.subtract)
                nc.vector.tensor_scalar(out=var[:], in0=var[:], scalar1=BN_EPS,
                                        scalar2=None, op0=AO.add)
                sd = pool.tile([128, NS], F32, tag="sd")
                nc.scalar.activation(out=sd[:], in_=var[:], func=AF.Sqrt)
                rstd = pool.tile([128, NS], F32, tag="rstd")
                nc.vector.reciprocal(out=rstd[:], in_=sd[:])
                scale = pool.tile([128, NS], F32, tag="scale")
                nc.vector.tensor_tensor(out=scale[:], in0=gcol[:], in1=rstd[:],
                                        op=AO.mult)
                shift = pool.tile([128, NS], F32, tag="shift")
                nc.vector.tensor_tensor(out=shift[:], in0=mu[:], in1=scale[:],
                                        op=AO.mult)
                nc.vector.tensor_tensor(out=shift[:], in0=bcol[:], in1=shift[:],
                                        op=AO.subtract)
                return scale, shift

            if PH >= 3:
                with tc.tile_pool(name="bn1sb", bufs=1) as bnp, \
                     tc.tile_pool(name="d2ps", bufs=1, space="PSUM") as d2ps, \
                     tc.tile_pool(name="d2sb", bufs=1) as d2sb:
                    scale1, shift1 = bn_scale_shift(bn1_o, F1, 3, sb["bn1g"],
                                                    sb["bn1b"], bnp)
                    for (c0, c1) in chunks:
                        for s in range(3):
                            tmp = d2sb.tile([128, 512], F32, tag="hmt", bufs=2,
                                            name="tmp")
                            nc.vector.scalar_tensor_tensor(
                                out=tmp[:, 0:c1 - c0], in0=xgT[:, s, c0:c1],
                                scalar=scale1[:, s:s + 1],
                                in1=sb["xTb"][:, s, c0:c1],
                                op0=AO.mult, op1=AO.add)
                            nc.scalar.activation(out=hmidT[:, s, c0:c1],
                                                 in_=tmp[:, 0:c1 - c0],
                                                 func=AF.Relu,
                                                 bias=shift1[:, s:s + 1])
                        for nt0 in range(c0, c1, 128):
                            nt1 = min(nt0 + 128, c1)
                            R = nt1 - nt0
                            b = nt0 // 128
                            p2 = d2ps.tile([128, 258], F32, tag="p2", bufs=3,
                                           name="p2")
                            for k in range(3):
                                nc.tensor.matmul(out=p2[:R, :],
                                                 lhsT=hmidT[:, k, nt0:nt1],
                                                 rhs=sb["w5f"][:, k, :],
                                                 start=(k == 0), stop=(k == 2))
                            hxt = d2sb.tile([128, ROW2], BF16, tag="hxt2",
                                            bufs=3, name="hxt")
                            nc.vector.tensor_tensor(out=hxt[:R, 0:F2 + 1],
                                                    in0=p2[:R, 0:F2 + 1],
                                                    in1=sb["b5rep"][:R, 0:F2 + 1],
                                                    op=AO.add)
                            nc.vector.memset(hxt[:R, F2 + 1:F2 + 2], 1.0)
                            nc.vector.memset(hxt[:R, F2 + 2:ROW2], 0)
                            nc.sync.dma_start(out=hx2_b[nt0:nt1, :],
                                              in_=hxt[:R, :])
                            nc.vector.tensor_tensor(out=adst2o[:R, b, :],
                                                    in0=p2[:R, 257:258],
                                                    in1=sb["b5rep"][:R, 257:258],
                                                    op=AO.add)
                    ag_chunk(hx2_b, hx2_f, 0 if not CHUNK_AG else RSP, NPC)
                    # z5 (residual path) — runs in the AllGather shadow
                    def emit_z5(c0, c1):
                        W = c1 - c0
                        for m in range(2):
                            pz = d2ps.tile([128, 512], F32, tag="pz5", bufs=2,
                                           name="pz5")
                            for k in range(3):
                                nc.tensor.matmul(
                                    out=pz[:, :W],
                                    lhsT=sb["w5"][:, k, m * 128:(m + 1) * 128],
                                    rhs=hmidT[:, k, c0:c1],
                                    start=(k == 0), stop=(k == 2))
                            nc.scalar.activation(out=z5T[:, m, c0:c1],
                                                 in_=pz[:, :W],
                                                 func=AF.Identity,
                                                 bias=sb["b5c"][:, m:m + 1])
                    for (c0, c1) in chunks:
                        emit_z5(c0, c1)

            if PH >= 4:
                stats2 = pp.tile([1, 2 * F2], F32, tag="stats2")
                sparse_phase(2, hx2_f, xg2T, adst2o, stats2, ga=2)
                nc.sync.dma_start(out=bn2_i[:], in_=stats2[:])
                nc.gpsimd.collective_compute("AllGather", AO.bypass,
                                             ins=[bn2_i[:].opt()],
                                             outs=[bn2_o[:].opt()],
                                             replica_groups=rg)

            # ================= Phase: BN2 + fc2 -> hout, with the train-edge
            # head's gather descriptors prepared in this window (prepare_only)
            NEC = TEC // EC
            with tc.tile_pool(name="tps", bufs=1, space="PSUM") as tps, \
                 tc.tile_pool(name="tsb", bufs=1) as tsb:
                HEAD_PREP = os.environ.get("BASS_GAT_HEADPREP", "1") == "1"
                NPREP = int(os.environ.get("BASS_GAT_NPREP", str(TEC // EC))) \
                    if HEAD_PREP else 0

                if PH >= 5:
                    with tc.tile_pool(name="bn2sb", bufs=1) as bnp2, \
                         tc.tile_pool(name="d3ps", bufs=1, space="PSUM") as d3ps, \
                         tc.tile_pool(name="d3sb", bufs=1) as d3sb:
                        scale2, shift2 = bn_scale_shift(bn2_o, F2, 2,
                                                        sb["bn2g"],
                                                        sb["bn2b"], bnp2)
                        for (c0, c1) in chunks:
                            for s in range(2):
                                tmp = d3sb.tile([128, 512], F32, tag="hft",
                                                bufs=2, name="tmp")
                                nc.vector.scalar_tensor_tensor(
                                    out=tmp[:, 0:c1 - c0],
                                    in0=xg2T[:, s, c0:c1],
                                    scalar=scale2[:, s:s + 1],
                                    in1=z5T[:, s, c0:c1],
                                    op0=AO.mult, op1=AO.add)
                                nc.scalar.activation(out=hfinT[:, s, c0:c1],
                                                     in_=tmp[:, 0:c1 - c0],
                                                     func=AF.Relu,
                                                     bias=shift2[:, s:s + 1])
                            for nt0 in range(c0, c1, 128):
                                nt1 = min(nt0 + 128, c1)
                                R = nt1 - nt0
                                po = d3ps.tile([128, F2], F32, tag="po", bufs=2,
                                               name="po")
                                for k in range(2):
                                    nc.tensor.matmul(out=po[:R, :],
                                                     lhsT=hfinT[:, k, nt0:nt1],
                                                     rhs=sb["w2f"][:, k, :],
                                                     start=(k == 0),
                                                     stop=(k == 1))
                                hob = d3sb.tile([128, F2], BF16, tag="hob",
                                                bufs=2, name="hob")
                                nc.vector.tensor_tensor(out=hob[:R, :],
                                                        in0=po[:R, :],
                                                        in1=sb["b2rep"][:R, :],
                                                        op=AO.add)
                                nc.sync.dma_start(out=ho_b[nt0:nt1, :],
                                                  in_=hob[:R, :])
                    ag_chunk(ho_b, ho_f, 0 if not CHUNK_AG else RSP, NPC)

                # ================= Phase: train-edge head
                if PH >= 6:
                    gabs = {}
                    if HEAD_PREP:
                        hd_sems = [nc.alloc_semaphore(f"hd_q{q}")
                                   for q in range(4)]
                        for ec in range(NPREP):
                            gab = tsb.tile([128, 2 * EC // 128, F2], BF16,
                                           tag="gab", bufs=NEC, name="gab")
                            ic0 = ec * (2 * EC // 16)
                            for j in range(2):
                                q = (2 * ec + j) % 4
                                nc.gpsimd.dma_gather(
                                    gab[:, 8 * j:8 * (j + 1), :], ho_f[:],
                                    sb["c_idx"][:, ic0 + 64 * j:
                                                ic0 + 64 * (j + 1)],
                                    num_idxs=EC, num_idxs_reg=EC,
                                    elem_size=F2, single_packet=True,
                                    queue_num=q, prepare_only=True,
                                    sem=hd_sems[q])
                            gabs[ec] = gab
                        for q in range(4):
                            nc.gpsimd.trigger_dma(count=None, queue_num=q)
                    for ec in range(NEC):
                        if HEAD_PREP and ec < NPREP:
                            gab = gabs.pop(ec)
                            # explicit data-landing waits: prep-mode reads are
                            # not ordered on DMA completion by Tile
                            nround = ec // 2 + 1
                            nc.vector.wait_ge(hd_sems[(2 * ec) % 4],
                                              16 * nround)
                            nc.vector.wait_ge(hd_sems[(2 * ec + 1) % 4],
                                              16 * nround)
                        else:
                            gab = tsb.tile([128, 2 * EC // 128, F2], BF16,
                                           tag="gab",
                                           bufs=(NEC if HEAD_PREP else 3),
                                           name="gab")
                            ic0 = ec * (2 * EC // 16)
                            for j in range(2):
                                q = (2 * ec + j) % 4
                                nc.gpsimd.dma_gather(
                                    gab[:, 8 * j:8 * (j + 1), :], ho_f[:],
                                    sb["c_idx"][:, ic0 + 64 * j:
                                                ic0 + 64 * (j + 1)],
                                    num_idxs=EC, num_idxs_reg=EC,
                                    elem_size=F2, single_packet=True,
                                    queue_num=q)
                        prn = tsb.tile([128, EC // 128, F2], BF16, tag="prn",
                                       bufs=2)
                        EH = EC // 256
                        for ph in range(2):
                            nc.vector.tensor_tensor(
                                out=prn[:, ph * EH:(ph + 1) * EH, :],
                                in0=gab[:, ph * EH:(ph + 1) * EH, :],
                                in1=gab[:, EC // 128 + ph * EH:
                                        EC // 128 + (ph + 1) * EH, :],
                                op=AO.mult)
                        prT = tsb.tile([128, 2, EC], BF16, tag="prT", bufs=2)
                        for et in range(EC // 128):
                            ptx = tps.tile([128, 2, 128], BF16, tag="ptt",
                                           bufs=2)
                            for s in range(2):
                                nc.tensor.transpose(
                                    out=ptx[:, s, :],
                                    in_=prn[:, et, s * 128:(s + 1) * 128],
                                    identity=eye_b[:])
                            if et % 2 == 0:
                                nc.vector.tensor_copy(
                                    out=prT[:, :, et * 128:(et + 1) * 128],
                                    in_=ptx[:])
                            else:
                                nc.scalar.activation(
                                    out=prT[:, :, et * 128:(et + 1) * 128],
                                    in_=ptx[:], func=AF.Identity)
                        ot = tsb.tile([7, EC], F32, tag="ot", bufs=2)
                        for et in range(EC // 512):
                            po = tps.tile([7, 512], F32, tag="pot", bufs=2)
                            for k in range(2):
                                nc.tensor.matmul(
                                    out=po[:, :], lhsT=w4b[:, k, :],
                                    rhs=prT[:, k, et * 512:(et + 1) * 512],
                                    start=(k == 0), stop=(k == 1))
                            nc.scalar.activation(
                                out=ot[:, et * 512:(et + 1) * 512],
                                in_=po[:, :], func=AF.Identity,
                                bias=sb["b4c"][:, 0:1])
                        nc.sync.dma_start(out=out_t[:, ec * EC:(ec + 1) * EC],
                                          in_=ot[:, :])
            if PH < 6:
                _early_out()

    nc.compile()
    return nc


def _make_in_maps(prep):
    shared = prep["shared"]
    maps = []
    for c in range(NCORES):
        m = {}
        for k in ("b1rep", "b5rep", "b5c", "b2rep", "w4", "b4c",
                  "bn1g", "bn1b", "bn2g", "bn2b",
                  "eye", "ones_col"):
            m[k] = np.ascontiguousarray(shared[k].astype(np.float32))
        for k in ("w1f", "w5f", "w5", "w2f", "iota_tpb"):
            m[k] = np.ascontiguousarray(shared[k].astype(ml_dtypes.bfloat16))
        cd = prep["cores"][c]
        m["xTb"] = np.ascontiguousarray(cd["xT"].astype(ml_dtypes.bfloat16))
        m["hx_idx"] = np.ascontiguousarray(cd["hx_idx"])
        m["padmask"] = np.ascontiguousarray(cd["padmask"])
        m["c_idx"] = np.ascontiguousarray(cd["c_idx"])
        m["dloc_col"] = np.ascontiguousarray(cd["dloc_col"])
        m["Stb"] = cd["Stb"]
        maps.append(m)
    return maps


def _ensure_ntff_hook():
    """Register the NTFF profile hook (missing antenv.axon_hooks shim)."""
    import sys, types
    if "antenv.axon_hooks" not in sys.modules:
        mod = types.ModuleType("antenv.axon_hooks")
        _h = [None]
        mod.set_axon_ntff_profile_hook = lambda h: _h.__setitem__(0, h)
        mod.get_axon_ntff_profile_hook = lambda: _h[0]
        sys.modules["antenv.axon_hooks"] = mod
        import antenv
        antenv.axon_hooks = mod
    import antenv.axon_hooks as ah
    if ah.get_axon_ntff_profile_hook() is None:
        try:
            from trn_agent_boot.trn_boot import _ntff_profile_via_ctypes
            ah.set_axon_ntff_profile_hook(
                _ntff_profile_via_ctypes("/opt/axon/libaxon_pjrt.so"))
        except Exception:
            pass


def kernel(**inputs):
    global last_exec_time_ns
    prep = _prepare(inputs)
    TPB = prep["TPB"]
    if TPB not in _PROG_CACHE:
        _PROG_CACHE[TPB] = _build_program(TPB)
    nc = _PROG_CACHE[TPB]
    in_maps = _make_in_maps(prep)
    trace = os.environ.get("BASS_GAT_TRACE", "0") == "1"
    if trace:
        _ensure_ntff_hook()
    res = run_bass_kernel_spmd(nc, in_maps, core_ids=list(range(NCORES)),
                               trace=trace)
    if trace:
        last_exec_time_ns = res.exec_time_ns
    out = np.concatenate(
        [res.results[c]["out_t"].T for c in range(NCORES)], axis=0)
    return out.astype(np.float32)
